# revision 68
# baseline (speedup 1.0000x reference)
"""Trainium2 Bass kernel for nn_EnhancedGraphConv (gnn_message_passing).

v5: v3 dataflow (host-gathered feature-major bf16 streams, degree-sorted
variable-width groups) restructured as a global chunk-granular software
pipeline:
 - at timestep tau, stage k runs on chunk tau-k, so PE/Act/DVE/Pool see a
   mix of all six stages at every moment (no per-stage engine bursts);
   per-group tails (softmax, message chunks, output MLP) drain one slice
   per timestep from queues,
 - sigmoid gates via tanh (same activation table as the softmax exp, so
   exactly one LoadActFuncSet): tn = (0.5Wn@xj+0.5bn)*(1+tanh((z+bg2)/2)),
 - softmax without max-subtraction (scores are O(1); padded slots carry
   -1e30 so exp underflows to 0); row sums via the Exp accum_out,
 - message sums via PE matmul accumulation into PSUM (Wc1 applied per
   neighbor-slot block) instead of wide DVE TensorReduce,
 - PSUM readers only on Act/DVE (HW GpSimd cannot touch PSUM); all
   SBUF-only small ops (masked-add, 1+tanh, rhsq, wrow, reshapes) on the
   otherwise idle Pool engine; I32r/identity built on device,
 - exec-order: smallish group first (short DMA ramp), widest next,
   smallest last (short final flush); weights split so chunk-0's arrive
   in the first DMA.
reps>1 (timing builds) repeats the body to measure steady-state exec with
the axon RPC floor cancelled.
"""
import numpy as np
import ml_dtypes
from contextlib import ExitStack

import concourse.bass as bass
import concourse.bacc as bacc
import concourse.tile as tile
from concourse import mybir
from concourse.bass_utils import run_bass_kernel_spmd
from concourse.masks import make_identity

F32 = mybir.dt.float32
BF16 = mybir.dt.bfloat16
NPBF = ml_dtypes.bfloat16
AF = mybir.ActivationFunctionType
OP = mybir.AluOpType

B, N, C, O, E = 2, 640, 64, 64, 18
D = 64            # max neighbor slots per row
RG = 32           # rows per group
NCORES = 8
RPC = (B * N) // NCORES   # 160 rows per core
NG = RPC // RG            # 5 groups
CHUNK = 512               # drain chunk (one PSUM bank)
MM = 512                  # matmul moving span (one PSUM bank)

_WSPEC = [
    ("We1", E, 64), ("We2", 64, 64), ("We3", 64, 32), ("Wxi", 64, 64),
    ("Ws", 64, 64), ("Wpe", 32, 128), ("Wjj", 64, 128), ("Wn", 64, 64),
    ("W22", 128, 128), ("Wa3", 32, 1),
    ("Wc1s", 64, 64), ("Wc1m", 64, 64), ("Wc2", 64, 64),
]
WPA = 160 + 128   # split col: We1..Ws in part A (chunk-0 + precompute)
_FSPEC = [
    ("be1", 64, 1), ("be2", 64, 1), ("be3", 32, 1), ("bhg", 128, 1),
    ("bn", 64, 1), ("ba2", 32, 1), ("bg2h", 64, 1), ("bs", 64, 1),
    ("bc1", 64, 1), ("bc2", 64, 1),
]


def _layout(spec):
    off, c = {}, 0
    for n, r, w in spec:
        off[n] = (r, c, w)
        c += w
    return off, c


_WOFF, WCOLS = _layout(_WSPEC)
_FOFF, FCOLS = _layout(_FSPEC)


def _build_nc(groups):
    """groups: exec-ordered tuples (dg, r0, pair). Row block rb = r0 // RG."""
    offs = np.concatenate([[0], np.cumsum([RG * dg for dg, _, _ in groups])])
    TOT = int(offs[-1])
    nc = bacc.Bacc("TRN2", target_bir_lowering=False)
    t = {}
    t["wp"] = nc.dram_tensor("wp", [128, WCOLS], BF16, kind="ExternalInput")
    t["fp"] = nc.dram_tensor("fp", [128, FCOLS], F32, kind="ExternalInput")
    t["xj"] = nc.dram_tensor("xj", [C, TOT], BF16, kind="ExternalInput")
    t["ef"] = nc.dram_tensor("ef", [E, TOT], BF16, kind="ExternalInput")
    t["xrf"] = nc.dram_tensor("xrf", [C, RPC], BF16, kind="ExternalInput")
    t["am"] = nc.dram_tensor("am", [RPC, D], F32, kind="ExternalInput")
    t["out"] = nc.dram_tensor("out", [RPC, O], F32, kind="ExternalOutput")

    with tile.TileContext(nc) as tc, ExitStack() as ctx:
        w = ctx.enter_context(tc.tile_pool(name="w", bufs=1))
        io = ctx.enter_context(tc.tile_pool(name="io", bufs=3))
        mlp = ctx.enter_context(tc.tile_pool(name="mlp", bufs=3))
        sm = ctx.enter_context(tc.tile_pool(name="sm", bufs=3))
        ps = ctx.enter_context(tc.tile_pool(name="ps", bufs=6, space="PSUM"))
        psc = ctx.enter_context(tc.tile_pool(name="psc", bufs=1, space="PSUM"))

        ident = w.tile([128, 128], F32)
        make_identity(nc, ident[:])
        i32r = w.tile([RG, CHUNK], BF16, name="i32r")
        i32_src = bass.AP(tensor=ident.tensor, offset=ident[:RG, :RG].offset,
                          ap=[ident[:RG].ap[0], [0, CHUNK // RG], [1, RG]])
        nc.gpsimd.tensor_copy(out=i32r[:], in_=i32_src)
        ones32 = w.tile([RG, 64], BF16, name="ones32")
        nc.gpsimd.memset(ones32[:], 1.0)

        wp = w.tile([128, WCOLS], BF16, name="wp")
        nc.sync.dma_start(out=wp[:, :WPA], in_=t["wp"][:, :WPA])
        fp = w.tile([128, FCOLS], F32, name="fp")
        nc.sync.dma_start(out=fp[:], in_=t["fp"][:])

        def wv(name):
            r, c0, cw = _WOFF[name]
            return wp[:r, c0:c0 + cw]

        def fv(name):
            r, c0, cw = _FOFF[name]
            return fp[:r, c0:c0 + cw]

        xrf = w.tile([C, RPC], BF16, name="xrf")
        nc.sync.dma_start(out=xrf[:], in_=t["xrf"][:])
        axi = w.tile([C, RPC], F32, name="axi")
        selff = w.tile([C, RPC], BF16, name="selff")
        axiT = w.tile([RG, NG * C], BF16, name="axiT")

        def emit_precompute():
            # per-node precomputes, emitted first: they fill the DMA ramp
            pa = ps.tile([C, RPC], F32, name="mlp")
            nc.tensor.matmul(pa[:], wv("Wxi"), xrf[:], start=True, stop=True)
            nc.vector.tensor_copy(out=axi[:], in_=pa[:])
            pb = ps.tile([C, RPC], F32, name="mlp")
            nc.tensor.matmul(pb[:], wv("Ws"), xrf[:], start=True, stop=True)
            nc.vector.tensor_scalar_add(out=selff[:], in0=pb[:], scalar1=fv("bs"))
            ptb = ps.tile([RG, NG * C], F32, name="mlp")
            for gg in range(NG):
                nc.tensor.transpose(ptb[:, gg * C:(gg + 1) * C],
                                    axi[:, gg * RG:(gg + 1) * RG],
                                    ident[:C, :C])
            nc.vector.tensor_copy(out=axiT[:], in_=ptb[:])

        # ---- global chunk-granular software pipeline ----
        # Stages per 512-token chunk: s1 We1/pe1, s2 We2/pe2, s3 We3/pe3,
        # s4 hg, s5 W22/h2/eq/scores, s6 Wn/tn.  At timestep tau, stage k
        # runs on chunk tau-k, so every engine sees a mix of stages at any
        # moment; per-group tails (softmax, message chunks, output MLP) are
        # queued and drained one slice per timestep.
        G = [None] * len(groups)

        def ensure_group(g):
            if G[g] is not None:
                return
            dg, r0, pair = groups[g]
            TGg = RG * dg
            gc = slice(int(offs[g]), int(offs[g + 1]))
            d = dict(g=g, dg=dg, r0=r0, rb=r0 // RG, pair=pair, TGg=TGg)
            d["chunks"] = [(s, min(s + CHUNK, TGg))
                           for s in range(0, TGg, CHUNK)]
            d["efg"] = io.tile([E, TGg], BF16, name="efg")
            nc.sync.dma_start(out=d["efg"][:], in_=t["ef"][:, gc])
            d["xjg"] = io.tile([C, TGg], BF16, name="xjg")
            nc.sync.dma_start(out=d["xjg"][:], in_=t["xj"][:, gc])
            d["amg"] = sm.tile([RG, dg], F32, name="amg")
            nc.sync.dma_start(out=d["amg"][:], in_=t["am"][r0:r0 + RG, :dg])
            for nm, rr in (("pe1", 64), ("pe2", 64), ("pe3", 32),
                           ("hg", 128), ("tn", 64), ("h2", 32)):
                d[nm] = mlp.tile([rr, TGg], BF16, name=nm)
            d["eq"] = mlp.tile([64, TGg], F32, name="eq")
            d["nsc"] = TGg // 128
            d["psc4"] = psc.tile([128, d["nsc"]], F32, name="sc")
            nseg = 2 if pair else 1
            sw = dg // nseg
            d["segs"] = [(k * sw, (k + 1) * sw) for k in range(nseg)]
            G[g] = d

        def _mm_spans(s, e):
            return [(hs, min(hs + MM, e)) for hs in range(s, e, MM)]

        def s1(d, s, e):
            p = ps.tile([128, CHUNK], F32, name="mlp")
            for hs, he in _mm_spans(s, e):
                nc.tensor.matmul(p[:64, hs - s:he - s], wv("We1"),
                                 d["efg"][:, hs:he], start=True, stop=True)
            nc.vector.tensor_scalar(out=d["pe1"][:, s:e], in0=p[:64, :e - s],
                                    scalar1=fv("be1"), scalar2=0.0,
                                    op0=OP.add, op1=OP.max)

        def s2(d, s, e):
            p = ps.tile([128, CHUNK], F32, name="mlp")
            for hs, he in _mm_spans(s, e):
                nc.tensor.matmul(p[:64, hs - s:he - s], wv("We2"),
                                 d["pe1"][:, hs:he], start=True, stop=True)
            nc.scalar.activation(d["pe2"][:, s:e], p[:64, :e - s], AF.Relu,
                                 bias=fv("be2"))

        def s3(d, s, e):
            p = ps.tile([128, CHUNK], F32, name="mlp")
            for hs, he in _mm_spans(s, e):
                nc.tensor.matmul(p[:32, hs - s:he - s], wv("We3"),
                                 d["pe2"][:, hs:he], start=True, stop=True)
            nc.vector.tensor_scalar(out=d["pe3"][:, s:e], in0=p[:32, :e - s],
                                    scalar1=fv("be3"), scalar2=0.0,
                                    op0=OP.add, op1=OP.max)

        def s4(d, s, e):
            # hg = relu(Wpe@pe3 + Wjj@xj + axi(row) + bhg); axi added via
            # matmul with replicated 32-identity rhs selecting the row block.
            p = ps.tile([128, CHUNK], F32, name="mlp")
            for hs, he in _mm_spans(s, e):
                sub = 1 if (d["pair"] and hs >= d["TGg"] // 2) else 0
                axiTg = axiT[:, (d["rb"] + sub) * C:(d["rb"] + sub + 1) * C]
                o = hs - s
                nc.tensor.matmul(p[:, o:o + he - hs], wv("Wjj"),
                                 d["xjg"][:, hs:he], start=True, stop=False)
                nc.tensor.matmul(p[:64, o:o + he - hs], axiTg,
                                 i32r[:, :he - hs], start=False, stop=False)
                nc.tensor.matmul(p[:, o:o + he - hs], wv("Wpe"),
                                 d["pe3"][:, hs:he], start=False, stop=True)
            nc.scalar.activation(d["hg"][:, s:e], p[:, :e - s], AF.Relu,
                                 bias=fv("bhg"))

        def s5(d, s, e, alt=False):
            # h2 = relu(Wa2.T@h1+ba2); gates via tanh (same act table as
            # exp): eq = 1 + tanh((z+bg2)/2) == 2*sigmoid(z+bg2);
            # the 0.5 factor is folded into Wn/bn on the host.
            # per-128 attention score matmuls as each h2 slice lands
            p = ps.tile([128, CHUNK], F32, name="mlp")
            for hs, he in _mm_spans(s, e):
                nc.tensor.matmul(p[:, hs - s:he - s], wv("W22"),
                                 d["hg"][:, hs:he], start=True, stop=True)
            if alt:
                nc.vector.tensor_scalar(out=d["h2"][:, s:e],
                                        in0=p[:32, :e - s],
                                        scalar1=fv("ba2"), scalar2=0.0,
                                        op0=OP.add, op1=OP.max)
            else:
                nc.scalar.activation(d["h2"][:, s:e], p[:32, :e - s],
                                     AF.Relu, bias=fv("ba2"))
            nc.scalar.activation(d["eq"][:, s:e], p[64:128, :e - s],
                                 AF.Tanh, bias=fv("bg2h"), scale=0.5)
            for c in range(s // 128, e // 128):
                nc.tensor.matmul(d["psc4"][:, c:c + 1],
                                 d["h2"][:, c * 128:(c + 1) * 128],
                                 wv("Wa3"), start=True, stop=True)
            nc.gpsimd.tensor_scalar_add(out=d["eq"][:, s:e],
                                        in0=d["eq"][:, s:e], scalar1=1.0)

        def s6(d, s, e):
            # tn = (0.5*Wn@xj + 0.5*bn) * (1 + tanh((z+bg2)/2))
            #    == (Wn@xj + bn) * sigmoid(z + bg2)
            p = ps.tile([128, CHUNK], F32, name="mlp")
            for hs, he in _mm_spans(s, e):
                nc.tensor.matmul(p[:64, hs - s:he - s], wv("Wn"),
                                 d["xjg"][:, hs:he], start=True, stop=True)
            nc.vector.scalar_tensor_tensor(
                out=d["tn"][:, s:e], in0=p[:64, :e - s], scalar=fv("bn"),
                in1=d["eq"][:, s:e], op0=OP.add, op1=OP.mult)

        def softmax_tail(d):
            # exp without max subtraction: scores are O(1) and padded slots
            # are -1e30 -> exp == 0.  accum_out gives row sums.
            dg, nsc = d["dg"], d["nsc"]
            scb = sm.tile([128, nsc], F32, name="scb")
            nc.vector.tensor_copy(out=scb[:], in_=d["psc4"][:])
            sc_rm = sm.tile([RG, dg], F32, name="sc_rm")
            for pb4 in range(4):
                outap = bass.AP(tensor=sc_rm.tensor,
                                offset=sc_rm[:, pb4:pb4 + 1].offset,
                                ap=[sc_rm[:].ap[0], [4, nsc]])
                nc.gpsimd.tensor_copy(
                    out=outap, in_=scb[pb4 * RG:(pb4 + 1) * RG, :])
            smg = sm.tile([RG, dg], F32, name="smg")
            nc.gpsimd.tensor_tensor(out=smg[:], in0=sc_rm[:],
                                    in1=d["amg"][:], op=OP.add)
            nseg = len(d["segs"])
            pexp = sm.tile([RG, dg + nseg], F32, name="pexp")
            for k, (a, b) in enumerate(d["segs"]):
                nc.scalar.activation(pexp[:, a:b], smg[:, a:b], AF.Exp,
                                     accum_out=pexp[:, dg + k:dg + k + 1])
            invz = sm.tile([RG, nseg], F32, name="invz")
            nc.gpsimd.tensor_scalar_add(out=invz[:],
                                        in0=pexp[:, dg:dg + nseg],
                                        scalar1=1e-30)
            nc.vector.reciprocal(out=invz[:], in_=invz[:])
            d["wrow"] = sm.tile([RG, dg], BF16, name="wrow")
            for k, (a, b) in enumerate(d["segs"]):
                nc.gpsimd.tensor_scalar_mul(out=d["wrow"][:, a:b],
                                            in0=pexp[:, a:b],
                                            scalar1=invz[:, k:k + 1])
            d["mdw"] = mlp.tile([64, d["TGg"]], BF16, name="mdw")
            d["rhsq"] = sm.tile([RG, d["TGg"]], BF16, name="rhsq")

        def msg_chunk(d, s, e):
            # rhsq[r, t] = wrow[r, t//32 + off] * delta(t%32 == r); then
            # pwb = ones32.T @ rhsq broadcasts w(t) over feature partitions.
            wrow, tn = d["wrow"], d["tn"]
            nd = (e - s) // RG
            wb_ap = bass.AP(
                tensor=wrow.tensor,
                offset=wrow[:, s // RG:s // RG + nd].offset,
                ap=[wrow[:].ap[0], [1, nd], [0, RG]])
            nc.gpsimd.tensor_tensor(out=d["rhsq"][:, s:e], in0=wb_ap,
                                    in1=i32r[:, :e - s], op=OP.mult)
            pwb = ps.tile([128, CHUNK], F32, name="mlp")
            for hs, he in _mm_spans(s, e):
                nc.tensor.matmul(pwb[:64, hs - s:he - s], ones32[:],
                                 d["rhsq"][:, hs:he], start=True, stop=True)
            nc.vector.tensor_tensor(out=d["mdw"][:, s:e], in0=tn[:, s:e],
                                    in1=pwb[:64, :e - s], op=OP.mult)

        def out_tail(d):
            # pc1 accumulates Wc1s@selff + sum_d Wc1m@mdw_block on PE
            r0, dg = d["r0"], d["dg"]
            nseg = len(d["segs"])
            nr = RG * nseg
            pc1 = ps.tile([64, nr], F32, name="mlp")
            dsub = dg // nseg
            for k in range(nseg):
                reg = pc1[:, k * RG:(k + 1) * RG]
                nc.tensor.matmul(reg, wv("Wc1s"),
                                 selff[:, r0 + k * RG:r0 + (k + 1) * RG],
                                 start=True, stop=False, skip_group_check=True)
                for dd in range(dsub):
                    db = k * dsub + dd
                    nc.tensor.matmul(reg, wv("Wc1m"),
                                     d["mdw"][:, db * RG:(db + 1) * RG],
                                     start=False, stop=(dd == dsub - 1),
                                     skip_group_check=True)
            c1 = sm.tile([64, nr], BF16, name="c1")
            nc.vector.tensor_scalar(out=c1[:], in0=pc1[:], scalar1=fv("bc1"),
                                    scalar2=0.0, op0=OP.add, op1=OP.max)
            pc2 = ps.tile([64, nr], F32, name="mlp")
            nc.tensor.matmul(pc2[:], wv("Wc2"), c1[:], start=True, stop=True)
            ofm = sm.tile([64, nr], F32, name="ofm")
            nc.vector.tensor_scalar_add(out=ofm[:], in0=pc2[:], scalar1=fv("bc2"))
            por = ps.tile([nr, 64], F32, name="mlp")
            nc.tensor.transpose(por[:], ofm[:], ident[:64, :64])
            orow = sm.tile([nr, 64], F32, name="orow")
            nc.vector.tensor_copy(out=orow[:], in_=por[:])
            nc.sync.dma_start(out=t["out"][r0:r0 + nr, :], in_=orow[:])

        per_g = []
        for g, (dg, r0, pair) in enumerate(groups):
            TGg = RG * dg
            chs = [(s, min(s + CHUNK, TGg)) for s in range(0, TGg, CHUNK)]
            per_g.append([(g, s, e, ci == len(chs) - 1)
                          for ci, (s, e) in enumerate(chs)])
        allch = [c for v in per_g for c in v]

        ensure_group(allch[0][0])
        nc.sync.dma_start(out=wp[:, WPA:], in_=t["wp"][:, WPA:])
        emit_precompute()
        T = len(allch)
        DEPTH = 6
        smx_done = [False] * len(groups)
        hold = [[] for _ in range(groups.__len__())]
        msgq = []
        msg_left = [len(G[g]["chunks"]) if G[g] else None
                    for g in range(len(groups))]
        msg_left = [len([1 for s in range(0, RG * dg, CHUNK)])
                    for dg, _, _ in groups]
        nmsg = 0
        for tau in range(T + DEPTH):
            if tau + 5 < T:
                ensure_group(allch[tau + 5][0])
            npop = 1 if tau < T - 2 else len(msgq)
            for _ in range(min(npop, len(msgq))):
                g, s, e = msgq.pop(0)
                msg_chunk(G[g], s, e)
                nmsg += 1
                msg_left[g] -= 1
                if msg_left[g] == 0:
                    out_tail(G[g])
            for k in range(DEPTH - 1, -1, -1):
                i = tau - k
                if not (0 <= i < T):
                    continue
                g, s, e, last = allch[i]
                d = G[g]
                if k == 0:
                    s1(d, s, e)
                elif k == 1:
                    s2(d, s, e)
                elif k == 2:
                    s3(d, s, e)
                elif k == 3:
                    s4(d, s, e)
                elif k == 4:
                    s5(d, s, e)
                    if last:
                        softmax_tail(d)
                        smx_done[g] = True
                        msgq.extend(hold[g])
                        hold[g] = []
                elif k == 5:
                    s6(d, s, e)
                    if smx_done[g]:
                        msgq.append((g, s, e))
                    else:
                        hold[g].append((g, s, e))
        while msgq:
            g, s, e = msgq.pop(0)
            msg_chunk(G[g], s, e)
            nmsg += 1
            msg_left[g] -= 1
            if msg_left[g] == 0:
                out_tail(G[g])
    nc.compile()
    return nc


_NC = None
_NC_KEY = None


def _host_prep(x, adjacency, edge_features, weights):
    """Build per-core input maps (sort by degree, gather + pack on host).

    Returns (in_maps, perms, dgs): perms[core] maps sorted position ->
    local row index within the core's 160 rows.
    """
    adj = adjacency > 0
    order = np.argsort(~adj, axis=-1, kind="stable")   # [B, N, N]
    deg = adj.sum(-1)                                  # [B, N]
    assert deg.max() <= D, f"degree {deg.max()} exceeds {D} slots"
    jidx = order[:, :, :D].astype(np.int64)            # [B, N, D]
    slot = np.arange(D)[None, None, :]
    valid = slot < deg[:, :, None]
    jidx = np.where(valid, jidx, 0)
    am = np.where(valid, 0.0, -1e30).astype(np.float32)  # [B, N, D]

    # per-core degree-descending row order; shared per-group slot widths
    perms = []
    dgs = np.zeros(NG, np.int64)
    for core in range(NCORES):
        b = core // 4
        i0 = (core % 4) * RPC
        p = np.argsort(-deg[b, i0:i0 + RPC], kind="stable")
        perms.append(p)
        sd = deg[b, i0:i0 + RPC][p]
        for g in range(NG):
            mx = int(sd[g * RG:(g + 1) * RG].max())
            dgs[g] = max(dgs[g], -(-mx // 4) * 4, 4)
    dgs = [int(v) for v in dgs]
    paired = (len(dgs) >= 2 and dgs[-1] <= RG and dgs[-2] <= RG)
    groups = []            # (dg, r0, pair); r0 = row base in sorted order
    rest = dgs[:-2] if paired else dgs
    if paired:             # merged: 64 rows on 32 partitions
        groups.append((2 * RG, (NG - 2) * RG, True))
    for gi, dg in enumerate(rest):
        groups.append((dg, gi * RG, False))
    # exec order: a small-ish group first (short startup DMA), widest in
    # the middle, smallest last (short final-flush tail)
    groups.sort(key=lambda q: -q[0])
    if len(groups) >= 3:
        groups = [groups[2], groups[0], groups[1]] + groups[3:]
    offs = np.concatenate([[0], np.cumsum([RG * dg for dg, _, _ in groups])])

    Wa1, Wg1 = weights["Wa1"], weights["Wg1"]
    W22 = np.zeros((128, 128), np.float32)
    W22[:64, :32] = weights["Wa2"]
    W22[64:, 64:] = weights["Wg2"]
    wvals = {
        "We1": weights["We1"], "We2": weights["We2"], "We3": weights["We3"],
        "Wpe": np.concatenate([Wa1[2 * C:], Wg1[C:]], 1),
        "Wjj": np.concatenate([Wa1[C:2 * C], Wg1[:C]], 1),
        "Wn": 0.5 * weights["Wn"], "W22": W22, "Wa3": weights["Wa3"],
        "Wc1s": weights["Wc1"][:64], "Wc1m": weights["Wc1"][64:],
        "Wc2": weights["Wc2"],
        "Wxi": Wa1[:C], "Ws": weights["Ws"],
    }
    fvals = {
        "be1": weights["be1"][:, None], "be2": weights["be2"][:, None],
        "be3": weights["be3"][:, None],
        "bhg": np.concatenate([weights["ba1"], weights["bg1"]])[:, None],
        "bn": 0.5 * weights["bn"][:, None], "ba2": weights["ba2"][:, None],
        "bg2h": 0.5 * weights["bg2"][:, None], "bs": weights["bs"][:, None],
        "bc1": weights["bc1"][:, None], "bc2": weights["bc2"][:, None],
    }
    wpk = np.zeros((128, WCOLS), NPBF)
    for name, (r, c0, cw) in _WOFF.items():
        v = np.asarray(wvals[name], np.float32)
        assert v.shape == (r, cw), (name, v.shape, (r, cw))
        wpk[:r, c0:c0 + cw] = v.astype(NPBF)
    fpk = np.zeros((128, FCOLS), np.float32)
    for name, (r, c0, cw) in _FOFF.items():
        v = np.asarray(fvals[name], np.float32)
        assert v.shape == (r, cw), (name, v.shape, (r, cw))
        fpk[:r, c0:c0 + cw] = v

    TOT = int(offs[-1])
    in_maps = []
    for core in range(NCORES):
        b = core // 4
        i0 = (core % 4) * RPC
        p = perms[core]
        jv = jidx[b, i0:i0 + RPC][p]                   # [RPC, D] sorted rows
        # token col = offs[g] + d*RG + r  (d-major per group, d < dgs[g])
        jcol = np.zeros(TOT, np.int64)
        lrow = np.zeros(TOT, np.int64)                 # sorted-local row
        for g, (dg, r0, pair) in enumerate(groups):
            if pair:
                blkA = jv[r0:r0 + RG, :RG]
                blkB = jv[r0 + RG:r0 + 2 * RG, :RG]
                blk = np.concatenate([blkA, blkB], axis=1)   # [RG, 64]
                lrA = np.broadcast_to(
                    np.arange(r0, r0 + RG)[None, :], (RG, RG))
                lrB = lrA + RG
                lr = np.concatenate([lrA, lrB], axis=0).reshape(-1)
            else:
                blk = jv[r0:r0 + RG, :dg]              # [RG, dg]
                lr = np.broadcast_to(
                    np.arange(r0, r0 + RG)[None, :],
                    (dg, RG)).reshape(-1)
            jcol[offs[g]:offs[g + 1]] = blk.T.reshape(-1)
            lrow[offs[g]:offs[g + 1]] = lr
        grow = i0 + p[lrow]                            # global row in batch b
        amc = am[b, i0:i0 + RPC][p]                    # [RPC, D] sorted
        amk = amc.copy()
        for dg, r0, pair in groups:
            if pair:
                amk[r0:r0 + RG, RG:2 * RG] = amc[r0 + RG:r0 + 2 * RG, :RG]
        amk = np.ascontiguousarray(amk, np.float32)
        m = {
            "wp": wpk, "fp": fpk,
            "xj": np.ascontiguousarray(x[b].T[:, jcol].astype(NPBF)),
            "ef": np.ascontiguousarray(
                edge_features[b, grow, jcol, :].T.astype(NPBF)),
            "xrf": np.ascontiguousarray(x[b, i0:i0 + RPC][p].T.astype(NPBF)),
            "am": amk,
        }
        in_maps.append(m)
    return in_maps, perms, tuple(groups)


def kernel(**inputs):
    global _NC, _NC_KEY
    x = np.asarray(inputs["x"], np.float32)
    adjacency = np.asarray(inputs["adjacency"], np.float32)
    edge_features = np.asarray(inputs["edge_features"], np.float32)
    weights = {k: np.asarray(v, np.float32) for k, v in inputs.items()
               if k not in ("x", "adjacency", "edge_features")}
    in_maps, perms, groups = _host_prep(
        x, adjacency, edge_features, weights)
    key = groups
    if _NC is None or _NC_KEY != key:
        _NC = _build_nc(groups)
        _NC_KEY = key
    res = run_bass_kernel_spmd(_NC, in_maps, list(range(NCORES)))
    out = np.zeros((B, N, O), np.float32)
    for core in range(NCORES):
        b = core // 4
        i0 = (core % 4) * RPC
        out[b, i0 + perms[core]] = res.results[core]["out"]
    return out
